# revision 1
# baseline (speedup 1.0000x reference)
"""Masked dot-product attention (B=16, S=4096, D=64) on 8 Trainium2 NeuronCores.

Decomposition: query-block sharding. Core c owns query rows [c*512, (c+1)*512)
of ALL batches. Every core runs the identical SPMD program: for each batch b it
loops over exactly kb[b] = ceil(valid_lens[b]/128) key-chunks (compile-time
constants derived from the valid_lens input on the host), so masked-out key
blocks are never computed and the load is perfectly balanced across cores.

Math (per batch b, per core c):
  S^T[k,q] = K_chunk[k,:] @ Q[q,:]^T / sqrt(D)      (TensorE, fp16, k on partitions)
  P^T      = exp(S^T)                               (ScalarE, no max-subtraction:
                                                     scores ~ N(0,1), no overflow)
  Oaug^T[65,q] += V_aug_chunk[k,:]^T @ P^T[k,q]     (TensorE, accumulate in PSUM)
where V_aug = [V | 1], with rows k >= valid_len zeroed on the host. The zeroed
rows make masking exact: masked keys contribute 0 to both the numerator and the
ones-column denominator. Host divides numerator by denominator at the end
(exactly softmax @ V, since exp(-1e6 + s) underflows to 0 in fp32 in the
reference as well).
"""

import numpy as np

import concourse.bacc as bacc
import concourse.tile as tile
from concourse import mybir
from concourse.bass_utils import run_bass_kernel_spmd

F16 = np.float16
F32 = np.float32

NCORES = 8
CH = 128   # key-chunk size (PSUM/PE partition dim)
EW = 65    # V_aug width: 64 value dims + 1 ones-column (softmax denominator)
# exp() groups alternate 4-chunk / 3-chunk tiles (7 PSUM banks total, ping-pong
# at tag granularity) + 1 bank for the PV accumulator = 8 banks exactly.
GRP_A, GRP_B = 4, 3


def _schedule(valid_lens, S):
    vl = np.asarray(valid_lens).astype(np.int64)
    vl = np.clip(vl, 1, S)
    kb = [int(-(-int(x) // CH)) for x in vl]          # ceil(valid/CH), >= 1
    pairs = [(x + 1) // 2 for x in kb]
    return vl, kb, pairs


def _slot_layout(kb, pairs, B, QB):
    # Three smallest slots first (tiny DMAs gate the first exps while the big
    # transfers stream in), then descending: big slots early keep compute
    # safely behind the DMA stream for the rest of the kernel.
    asc = sorted(range(B), key=lambda b: kb[b])
    lead = asc[:min(3, B)]
    slot_order = lead + asc[len(lead):][::-1]
    widths = [pairs[b] * CH + kb[b] * EW + QB for b in range(B)]
    boffs = {}
    off = 0
    for b in slot_order:
        boffs[b] = off
        off += widths[b]
    return slot_order, widths, boffs, off


def _build_program(kb, pairs, B, QB, D, slot_order, widths, boffs, blob_w):
    """Emit the SPMD Tile program. Identical on all cores; per-core data differs.

    Emission is software-pipelined one exp-group ahead: the PE's program order
    is S_0, S_1, PV_0, S_2, PV_1, ... so the scores for group j+1 are already
    in PSUM when exp(j) finishes — ScalarE (the bottleneck engine) never waits
    at slot boundaries.
    """
    dt = mybir.dt
    nc = bacc.Bacc(None, target_bir_lowering=False)

    # One DRAM blob per core: each slot's [kt | va | qt] columns concatenated,
    # so a slot needs exactly ONE input DMA (issue rate on the DMA queues was
    # the ramp limiter with 3 DMAs/slot).
    blob = nc.declare_dram_parameter("blob", [128, blob_w], dt.float16, False)
    oaug = nc.declare_dram_parameter("oaug", [B, EW, QB], dt.float32, True)

    # Flat group list with globally alternating 4/3 tags. Group sizes within a
    # slot are balanced (no 1-chunk tails): a tiny last group gives ScalarE a
    # short exp window while the PE still owes a full PV + next-scores
    # sequence, stalling the bottleneck engine at every slot boundary.
    groups = []  # (b, [chunk indices], tag, first_of_slot, last_of_slot)
    for b in slot_order:
        p0 = len(groups)
        caps, cap = [], 0
        while cap < kb[b]:
            caps.append(GRP_A if (p0 + len(caps)) % 2 == 0 else GRP_B)
            cap += caps[-1]
        excess = cap - kb[b]
        order = [j for j in range(len(caps)) if caps[j] == GRP_A] + \
                [j for j in range(len(caps)) if caps[j] == GRP_B]
        ti = 0
        while excess > 0:
            j = order[ti % len(order)]
            if caps[j] > 1:
                caps[j] -= 1
                excess -= 1
            ti += 1
        c0 = 0
        for gi, sz in enumerate(caps):
            groups.append((
                b, list(range(c0, c0 + sz)), (p0 + gi) % 2,
                gi == 0, gi == len(caps) - 1,
            ))
            c0 += sz

    with tile.TileContext(nc) as tc:
        with (
            tc.tile_pool(name="ins", bufs=1) as ins,
            tc.tile_pool(name="ptp", bufs=4) as ptp,
            tc.tile_pool(name="obp", bufs=4) as obp,
            tc.tile_pool(name="sca", bufs=1, space="PSUM") as scap,
            tc.tile_pool(name="scb", bufs=1, space="PSUM") as scbp,
            tc.tile_pool(name="acp", bufs=1, space="PSUM") as acp,
        ):
            # Tiny exp at the very top: forces the ~2.7us ACT table load to
            # overlap the first input DMAs instead of gating the first group.
            warm = ins.tile([128, 1], dt.float32, tag="warm")
            nc.vector.memset(warm[:], 0.0)
            nc.scalar.activation(
                warm[:], warm[:], mybir.ActivationFunctionType.Exp
            )

            # Two DMA segments per slot: [kt|qt] gates the slot's scores
            # matmuls, [va] is only needed ~1.5us later by the first PV.
            # Issue in earliest-deadline order (estimated compute need time)
            # so the transfer stream stays just ahead of ScalarE.
            # deadline = when compute first reads the segment; launch big
            # segments by (deadline - transfer time) so they land in time.
            need = {}
            cum = 0.0
            for b in slot_order:
                w1b = (pairs[b] * CH + QB) * 128 * 2      # seg1 bytes
                w2b = kb[b] * EW * 128 * 2                # seg2 bytes
                need[(b, 1)] = cum - w1b / 150e3          # 150 GB/s stream
                need[(b, 2)] = cum + 1.5 - w2b / 150e3
                cum += kb[b] * 0.507  # us of exp work per key-chunk
            dma_items = sorted(need, key=lambda x: need[x])

            tiles = {}
            for b in slot_order:
                tiles[b] = ins.tile(
                    [128, widths[b]], dt.float16, tag=f"blob{b}",
                    name=f"blob{b}",
                )
            kts, vas, qts = {}, {}, {}
            for b in slot_order:
                t = tiles[b]
                w_kt = pairs[b] * CH
                kts[b] = t[:, :w_kt]
                qts[b] = t[:, w_kt:w_kt + QB]
                vas[b] = t[:, w_kt + QB:]
            dma_engines = [nc.sync, nc.gpsimd]
            for di, (b, seg) in enumerate(dma_items):
                w1 = pairs[b] * CH + QB
                if seg == 1:
                    dma_engines[di % 2].dma_start(
                        out=tiles[b][:, :w1],
                        in_=blob[:, boffs[b]:boffs[b] + w1],
                    )
                else:
                    dma_engines[di % 2].dma_start(
                        out=tiles[b][:, w1:],
                        in_=blob[:, boffs[b] + w1:boffs[b] + widths[b]],
                    )

            accs = {}

            def emit_scores(b, chunks, tag):
                cap = GRP_A if tag == 0 else GRP_B
                pool = scap if tag == 0 else scbp
                sc = pool.tile([128, cap, QB], dt.float32, tag="sc")
                for i, ci in enumerate(chunks):
                    pj, par = divmod(ci, 2)
                    lhsT = kts[b][par * 64:(par + 1) * 64, pj * CH:(pj + 1) * CH]
                    rhs = qts[b][par * 64:(par + 1) * 64, :]
                    nc.tensor.matmul(
                        sc[:, i, :], lhsT, rhs,
                        start=True, stop=True,
                        tile_position=(par * 64, 0),
                    )
                return sc

            def emit_pv(b, chunks, pt, last_of_slot):
                if chunks[0] == 0:
                    accs[b] = acp.tile([128, QB], dt.float32, tag="acc", name="acc")
                for i, ci in enumerate(chunks):
                    nc.tensor.matmul(
                        accs[b][0:EW, :],
                        vas[b][:, ci * EW:(ci + 1) * EW],
                        pt[:, i, :],
                        start=(ci == 0),
                        stop=(ci == kb[b] - 1),
                    )
                if last_of_slot:
                    ob = obp.tile([128, QB], dt.float32, tag="ob")
                    nc.vector.tensor_copy(ob[0:EW, :], accs[b][0:EW, :])
                    nc.sync.dma_start(out=oaug[b], in_=ob[0:EW, :])

            prev = None  # (b, chunks, pt, last_of_slot)
            for b, chunks, tag, first, last in groups:
                sc = emit_scores(b, chunks, tag)
                if prev is not None:
                    emit_pv(prev[0], prev[1], prev[2], prev[3])
                n = len(chunks)
                cap = GRP_A if tag == 0 else GRP_B
                pt = ptp.tile([128, GRP_A, QB], dt.float16, tag="pt")
                nc.scalar.activation(
                    pt[:, :n, :], sc[:, :n, :],
                    mybir.ActivationFunctionType.Exp,
                    scale=float(1.0 / np.sqrt(D)),
                )
                prev = (b, chunks, pt, last)
            emit_pv(prev[0], prev[1], prev[2], prev[3])

    nc.compile()
    return nc


def _prepare(q, k, v, valid_lens):
    """Host-side sharding/layout. Returns (nc, in_maps, meta)."""
    q = np.asarray(q, dtype=F32)
    k = np.asarray(k, dtype=F32)
    v = np.asarray(v, dtype=F32)
    B, S, D = q.shape
    QB = S // NCORES
    vl, kb, pairs = _schedule(valid_lens, S)
    TOT, TP = sum(kb), sum(pairs)

    slot_order, widths, boffs, blob_w = _slot_layout(kb, pairs, B, QB)

    # Shared (core-independent) part of each slot's blob segment:
    #  kt [128, pairs*CH] fp16: pair j: partitions 0:64 <- K^T chunk 2j,
    #    64:128 <- K^T chunk 2j+1 (zero if absent) — concurrent row-group
    #    matmuls use both halves of the systolic array.
    #  va [128, kb*EW] fp16: va[p, g*EW + e] = V_aug[b, g*CH + p, e] with
    #    V_aug = [V | 1] and rows >= valid_len zeroed (exact masking).
    kT = np.ascontiguousarray(k.transpose(0, 2, 1)).astype(F16)  # [B, D, S]
    va_aug = np.zeros((B, S, EW), dtype=F32)
    va_aug[:, :, :D] = v
    va_aug[:, :, D] = 1.0
    for b in range(B):
        va_aug[b, int(vl[b]):, :] = 0.0
    va_aug = va_aug.astype(F16)

    # Slot segment layout: [kt | qt | va] (kt+qt gate scores; va gates PV).
    base = np.zeros((128, blob_w), dtype=F16)
    for b in range(B):
        o = boffs[b]
        for j in range(pairs[b]):
            c0, c1 = 2 * j, 2 * j + 1
            base[0:64, o + j * CH:o + (j + 1) * CH] = \
                kT[b][:, c0 * CH:(c0 + 1) * CH]
            if c1 < kb[b]:
                base[64:128, o + j * CH:o + (j + 1) * CH] = \
                    kT[b][:, c1 * CH:(c1 + 1) * CH]
        o = boffs[b] + pairs[b] * CH + QB
        blk = va_aug[b, :kb[b] * CH, :].reshape(kb[b], CH, EW)
        base[:, o:o + kb[b] * EW] = \
            blk.transpose(1, 0, 2).reshape(CH, kb[b] * EW)

    # Per-core part: Q^T slice for this core's query block, duplicated on both
    # partition halves (each PE row-group streams its own rhs copy).
    qT = np.ascontiguousarray(q.transpose(0, 2, 1)).astype(F16)  # [B, D, S]
    in_maps = []
    for c in range(NCORES):
        blob = base.copy()
        for b in range(B):
            o = boffs[b] + pairs[b] * CH
            sl = qT[b][:, c * QB:(c + 1) * QB]
            blob[0:64, o:o + QB] = sl
            blob[64:128, o:o + QB] = sl
        in_maps.append({"blob": blob})

    nc = _build_program(kb, pairs, B, QB, D, slot_order, widths, boffs, blob_w)
    return nc, in_maps, (B, S, D, QB)


def _postprocess(results, meta):
    B, S, D, QB = meta
    out = np.empty((B, S, D), dtype=F32)
    for c in range(NCORES):
        oa = results[c]["oaug"]          # [B, EW, QB] f32
        num = oa[:, :D, :]
        den = oa[:, D:D + 1, :]
        out[:, c * QB:(c + 1) * QB, :] = (num / den).transpose(0, 2, 1)
    return out


def kernel(q, k, v, valid_lens):
    nc, in_maps, meta = _prepare(q, k, v, valid_lens)
    try:
        res = run_bass_kernel_spmd(nc, in_maps, list(range(NCORES)))
    except Exception:
        # transient device errors (e.g. a wedged NeuronCore from an earlier
        # aborted run) usually clear on retry
        res = run_bass_kernel_spmd(nc, in_maps, list(range(NCORES)))
    return _postprocess(res.results, meta)

